# revision 60
# baseline (speedup 1.0000x reference)
"""GraphSAGE 2-layer minibatch kernel for 8 TRN2 NeuronCores.

Strategy: data-parallel over the 1024-target batch (128 targets/core).
The host lays out each core's working set as fp16 DRAM streams in
compute order, pre-transposed to feature-major: per block (block 0 =
targets, blocks 1..10 = the s2-major nb2 rows) a self tile
[128 feat-partitions, 2 feat-chunks x 128 rows] and a neighbor slab
[128, 26 slot units x 2 chunks x 128 rows] slot-major (padded with one
zero unit). Each slab moves as two DMAs: the first half over the HWDGE
queues (sync/scalar alternating), the second half as a GPSIMD
software-DGE DMA that accumulates (cce add) into the first, so the DMA
itself performs the first fold level (26 -> 13 units). The remaining
13-unit neighbor mean folds as a 5-op tree of wide contiguous DVE adds
(mean scale folded into pre-scaled fp16 weights).

The SAGE layer runs row-major: the feature-major data chunks are the
matmul's stationary lhsT and the fp16 weights the moving rhs, so
z = cat @ W.T lands as [row-partitions, H-cols] in f32 PSUM, the bias
joins as a rank-1 matmul (ones x b) in the same accumulation group,
and the L2 normalize runs per row on ACT: Relu from PSUM, Square with
accum_out (row sum-of-squares), Sqrt, DVE fast reciprocal, Copy with
per-partition scale. The per-block tail (reciprocal, scale,
agg2-accumulate) is software-pipelined one block behind the bulk work
so the in-order DVE/ACT queues never stall a block's fold/matmul on
the previous block's cross-engine round-trip. Layer 2 consumes
PE-transposed layer-1 outputs (block 0 = self half, running sum of
blocks 1..10 = agg half). All DMAs are issued up-front so the kernel
runs at HBM streaming bandwidth with compute chasing the stream.
"""

import numpy as np

N_NODES = 100000
D = 256
H = 256
B = 1024
S1 = 25
S2 = 10
NCORES = 8
BL = B // NCORES          # 128 rows per core
NBLK = 1 + S2             # 11 blocks of 128 layer-1 rows per core
P = 128
CH = D // P               # 2 feature chunks
U = CH * P                # 256-col slot unit (both chunks)
NSU = S1                  # 25 slot units
NBW = NSU * U             # 6400 neighbor cols per block in DRAM
HBW = NBW // 2            # 3200-col half-slab (one per HWDGE queue)
KC = 4                    # contraction chunks per layer (2*D/P)
HC = H // P               # 2 output-feature chunks

_PROG = None  # cached so repeat calls reuse the built program


def _build_program():
    import concourse.mybir as mybir
    from concourse.bacc import Bacc
    from concourse.masks import make_identity
    from concourse.tile import TileContext

    f32 = mybir.dt.float32
    f16 = mybir.dt.float16
    AF = mybir.ActivationFunctionType
    add_op = mybir.AluOpType.add

    nc = Bacc(trn_type="TRN2")

    self_d = nc.dram_tensor("selfs", (P, NBLK * U), f16, kind="ExternalInput")
    nb_d = nc.dram_tensor("nb", (NBLK * P, NBW), f16, kind="ExternalInput")
    w1c_d = nc.dram_tensor("w1c", (P, KC * H), f16, kind="ExternalInput")
    w2c_d = nc.dram_tensor("w2c", (P, KC * H), f16, kind="ExternalInput")
    b1r_d = nc.dram_tensor("b1r", (1, H), f16, kind="ExternalInput")
    b2r_d = nc.dram_tensor("b2r", (1, H), f16, kind="ExternalInput")
    zT_d = nc.dram_tensor("zT", (P, H), f32, kind="ExternalOutput")

    with TileContext(nc) as tc:
        with (
            tc.tile_pool(name="const", bufs=1) as cpool,
            tc.tile_pool(name="nbsg", bufs=S2 // 2) as nbspool,
            tc.tile_pool(name="scr", bufs=2) as scrpool,
            tc.tile_pool(name="agg", bufs=2) as apool,
            tc.tile_pool(name="zsb", bufs=3) as zpool,
            tc.tile_pool(name="sq", bufs=2) as sqpool,
            tc.tile_pool(name="nrm", bufs=3) as nrmpool,
            tc.tile_pool(name="h1", bufs=1) as h1pool,
            tc.tile_pool(name="mm_ps", bufs=2, space="PSUM") as mmpool,
            tc.tile_pool(name="l2_ps", bufs=1, space="PSUM") as l2pool,
            tc.tile_pool(name="tr_ps", bufs=2, space="PSUM") as trpool,
        ):
            ones16 = cpool.tile([1, P], f16, tag="ones16")
            nc.gpsimd.memset(ones16[:], 1.0)
            eps_sb = cpool.tile([P, 1], f32, tag="eps")
            nc.gpsimd.memset(eps_sb[:], 1e-8)
            ident16 = cpool.tile([P, P], f16, tag="ident16")
            make_identity(nc, ident16[:])

            # ---- stream: slab 0 leads; consts interleave behind it ---------
            # Each slab is split across both HWDGE queues. Weights/selfs are
            # queued behind slab 0 (needed from the first sage, ~10us in);
            # w2/b2 ride mid-stream (needed only at layer 2).
            w1_sb = cpool.tile([P, KC * H], f16, tag="w1")
            w2_sb = cpool.tile([P, KC * H], f16, tag="w2")
            b1_sb = cpool.tile([1, H], f16, tag="b1")
            b2_sb = cpool.tile([1, H], f16, tag="b2")
            selfs_sb = cpool.tile([P, NBLK * U], f16, tag="selfs")

            # block 0 rides alone; blocks 1..10 pair up so two blocks share
            # one fold tree. Four half-slab DMAs per pair, spread over the
            # queues (a few mid-stream halves go to the GPSIMD SWDGE queue).
            # One full-width DMA per slab (bigger transfers run the queues
            # nearer peak rate than half/quarter splits), one queue per
            # block within each pair.
            nb0_t = cpool.tile([P, NBW], f16, tag="nb0")
            nc.sync.dma_start(out=nb0_t[:, 0:HBW], in_=nb_d[0:P, 0:HBW])
            nc.scalar.dma_start(out=nb0_t[:, HBW:NBW], in_=nb_d[0:P, HBW:NBW])
            nc.sync.dma_start(out=w1_sb[:], in_=w1c_d[:])
            # selfs split across queues; with w2 on scalar both queues carry
            # exactly half the stream, so the last slab lands ~2us earlier
            SHLF = NBLK * U // 2
            nc.sync.dma_start(out=selfs_sb[:, 0:SHLF], in_=self_d[:, 0:SHLF])
            nc.scalar.dma_start(out=selfs_sb[:, SHLF:], in_=self_d[:, SHLF:])
            nc.sync.dma_start(out=b1_sb[:], in_=b1r_d[:])

            pairs = []
            for k in range(S2 // 2):  # blocks 1..10 in pairs
                a, b = 1 + 2 * k, 2 + 2 * k
                t = nbspool.tile([P, 2 * NBW], f16, tag="nbp", name=f"nbp{k}")
                e0 = nc.sync if k % 2 == 0 else nc.scalar
                e1 = nc.scalar if k % 2 == 0 else nc.sync
                e0.dma_start(out=t[:, 0:NBW], in_=nb_d[a * P:(a + 1) * P, :])
                e1.dma_start(out=t[:, NBW:2 * NBW],
                             in_=nb_d[b * P:(b + 1) * P, :])
                if k == 0:
                    nc.scalar.dma_start(out=w2_sb[:], in_=w2c_d[:])
                    nc.scalar.dma_start(out=b2_sb[:], in_=b2r_d[:])
                pairs.append(((a, b), t))

            h1tT_sb = h1pool.tile([P, H], f16, tag="h1tT")   # block-0, f-major
            agg2_sb = h1pool.tile([P, H], f16, tag="agg2")   # sum blocks 1..10
            agg2T_sb = h1pool.tile([P, H], f16, tag="agg2T")
            z2_sb = h1pool.tile([P, H], f32, tag="z2")
            h1t_sb = h1pool.tile([P, H], f16, tag="h1t")

            def fold(slab_t, nblks, split_op1=False):
                """Sum 25 slot units (256 cols each) per block with a wide
                DVE tree; nblks blocks fold in one tree via 3D APs (the
                inner runs stay >= 3072 cols, on the DVE fast path).
                Returns agg [P, nblks*U] feature-major, block j at
                [:, j*U:(j+1)*U]."""
                s3 = slab_t[:].rearrange("p (b w) -> p b w", b=nblks)
                u = lambda a, b: s3[:, :, a * U:b * U]
                scr = scrpool.tile([P, nblks * 12 * U], f16, tag="scr")
                c3 = scr[:].rearrange("p (b w) -> p b w", b=nblks)
                c = lambda a, b: c3[:, :, a * U:b * U]
                if split_op1:
                    # per-block first level: block j's half starts as soon
                    # as its own slab lands (the stream's last arrivals)
                    for j in range(nblks):
                        nc.vector.tensor_tensor(
                            out=c3[:, j:j + 1, 0:12 * U],
                            in0=s3[:, j:j + 1, 0:12 * U],
                            in1=s3[:, j:j + 1, 12 * U:24 * U],
                            op=add_op)
                else:
                    nc.vector.tensor_tensor(out=c(0, 12), in0=u(0, 12),
                                            in1=u(12, 24), op=add_op)
                nc.vector.tensor_tensor(out=c(0, 6), in0=c(0, 6),
                                        in1=c(6, 12), op=add_op)
                nc.vector.tensor_tensor(out=c(0, 3), in0=c(0, 3),
                                        in1=c(3, 6), op=add_op)
                agg_t = apool.tile([P, nblks * U], f16, tag="agg")
                a3 = agg_t[:].rearrange("p (b w) -> p b w", b=nblks)
                nc.vector.tensor_tensor(out=a3, in0=c(0, 1),
                                        in1=c(1, 2), op=add_op)
                nc.vector.tensor_tensor(out=a3, in0=a3,
                                        in1=c(2, 3), op=add_op)
                nc.vector.tensor_tensor(out=a3, in0=a3,
                                        in1=u(24, 25), op=add_op)
                return agg_t

            def sage_front(cat_chunks, w_sb, b_sb):
                """Bulk of the row-major SAGE layer: matmuls + Relu + row
                sum-of-squares + Sqrt. Returns (z_sb, n_t)."""
                z_ps = mmpool.tile([P, H], f32, space="PSUM", tag="mm")
                for k in range(KC):
                    nc.tensor.matmul(
                        out=z_ps[:],
                        lhsT=cat_chunks[k],
                        rhs=w_sb[:, k * H:(k + 1) * H],
                        start=(k == 0),
                        stop=False,
                    )
                # bias as a rank-1 accumulate: ones(rows) x b
                nc.tensor.matmul(
                    out=z_ps[:], lhsT=ones16[:], rhs=b_sb[:],
                    start=False, stop=True,
                )
                z_sb = zpool.tile([P, H], f16, tag="z")
                nc.scalar.activation(out=z_sb[:], in_=z_ps[:], func=AF.Relu)
                sq_sb = sqpool.tile([P, H], f16, tag="sq")
                ss_t = nrmpool.tile([P, 1], f32, tag="ss")
                nc.scalar.activation(out=sq_sb[:], in_=z_sb[:], func=AF.Square,
                                     accum_out=ss_t[:])
                # inv = (ssq + eps)^-1/2 in a single ACT op; eps keeps
                # all-zero rows finite (z * 1/n = 0 * 1e4 = 0).
                inv = nrmpool.tile([P, 1], f32, tag="inv")
                nc.scalar.activation(out=inv[:], in_=ss_t[:],
                                     func=AF.Abs_reciprocal_sqrt,
                                     bias=eps_sb[:])
                return z_sb, inv

            def sage_tail(src, z_sb, inv):
                """Deferred per-block tail: scale into the block's
                destination (+ agg2 accumulate / h1t transposes)."""
                if src == 0:
                    nc.scalar.activation(out=h1t_sb[:], in_=z_sb[:],
                                         func=AF.Copy, scale=inv[:])
                    transpose2(h1tT_sb, h1t_sb)  # early, off the tail
                elif src == 1:
                    nc.scalar.activation(out=agg2_sb[:], in_=z_sb[:],
                                         func=AF.Copy, scale=inv[:])
                else:
                    hn_t = zpool.tile([P, H], f16, tag="hn")
                    nc.scalar.activation(out=hn_t[:], in_=z_sb[:],
                                         func=AF.Copy, scale=inv[:])
                    nc.vector.tensor_tensor(
                        out=agg2_sb[:], in0=agg2_sb[:], in1=hn_t[:],
                        op=add_op,
                    )

            def transpose2(dst_sb, src_sb):
                """PE-transpose both [P, P] chunks of a row-major [P, H] tile
                into the feature-major dst."""
                for c in range(HC):
                    tr_ps = trpool.tile([P, P], f16, space="PSUM", tag="tr")
                    nc.tensor.transpose(
                        out=tr_ps[:],
                        in_=src_sb[:, c * P:(c + 1) * P],
                        identity=ident16[:],
                    )
                    nc.scalar.copy(dst_sb[:, c * P:(c + 1) * P], tr_ps[:])

            # ---- layer 1: blocks 0..10, tail skewed one block back ---------
            pend = None

            def do_block(src, agg_t, off):
                nonlocal pend
                cat = [
                    selfs_sb[:, src * U:src * U + P],
                    selfs_sb[:, src * U + P:(src + 1) * U],
                    agg_t[:, off:off + P],
                    agg_t[:, off + P:off + 2 * P],
                ]
                z_sb, n_t = sage_front(cat, w1_sb, b1_sb)
                if pend is not None:
                    sage_tail(*pend)
                pend = (src, z_sb, n_t)

            l2_ps = l2pool.tile([P, H], f32, space="PSUM", tag="l2")

            do_block(0, fold(nb0_t, 1), 0)
            for pi, ((a, b), t) in enumerate(pairs):
                agg_t = fold(t, 2, split_op1=True)
                do_block(a, agg_t, 0)
                do_block(b, agg_t, U)
                if pi == 1:
                    # pre-accumulate layer 2's self half + bias into a held
                    # PSUM bank, mid-stream (h1tT and w2 are resident by now)
                    for c in range(HC):
                        nc.tensor.matmul(
                            out=l2_ps[:],
                            lhsT=h1tT_sb[:, c * P:(c + 1) * P],
                            rhs=w2_sb[:, c * H:(c + 1) * H],
                            start=(c == 0),
                            stop=False,
                        )
                    nc.tensor.matmul(
                        out=l2_ps[:], lhsT=ones16[:], rhs=b2_sb[:],
                        start=False, stop=False,
                    )
            sage_tail(*pend)

            # ---- layer 2: only the agg half remains ------------------------
            transpose2(agg2T_sb, agg2_sb)
            for c in range(HC):
                nc.tensor.matmul(
                    out=l2_ps[:],
                    lhsT=agg2T_sb[:, c * P:(c + 1) * P],
                    rhs=w2_sb[:, (2 + c) * H:(3 + c) * H],
                    start=False,
                    stop=(c == HC - 1),
                )
            zf_sb = zpool.tile([P, H], f16, tag="z")
            nc.scalar.activation(out=zf_sb[:], in_=l2_ps[:], func=AF.Relu)
            sqf_sb = sqpool.tile([P, H], f16, tag="sq")
            ssf_t = nrmpool.tile([P, 1], f32, tag="ss")
            nc.scalar.activation(out=sqf_sb[:], in_=zf_sb[:], func=AF.Square,
                                 accum_out=ssf_t[:])
            invf = nrmpool.tile([P, 1], f32, tag="inv")
            nc.scalar.activation(out=invf[:], in_=ssf_t[:],
                                 func=AF.Abs_reciprocal_sqrt, bias=eps_sb[:])
            nc.scalar.activation(out=z2_sb[:], in_=zf_sb[:], func=AF.Copy,
                                 scale=invf[:])
            nc.sync.dma_start(out=zT_d[:], in_=z2_sb[:])

    nc.finalize()
    return nc


def _get_program():
    global _PROG
    if _PROG is None:
        _PROG = _build_program()
    return _PROG


def make_in_maps(x, targets, nb1_self, nb2, nb1_nb, W1, b1, W2, b2):
    """Host-side sharding/preprocessing -> per-core input dicts."""
    x = np.ascontiguousarray(np.asarray(x, dtype=np.float32))
    W1 = np.asarray(W1, dtype=np.float32)
    W2 = np.asarray(W2, dtype=np.float32)
    b1 = np.asarray(b1, dtype=np.float32)
    b2 = np.asarray(b2, dtype=np.float32)
    targets = np.asarray(targets).astype(np.int64)
    nb1_self = np.asarray(nb1_self).astype(np.int64)
    nb2 = np.asarray(nb2).astype(np.int64)
    nb1_nb = np.asarray(nb1_nb).astype(np.int64)

    # fold the neighbor-mean scale into the agg half of each weight matrix,
    # pre-chunked to the SBUF layout: w[p, k*H + m] = W.T[k*128 + p, m]
    def chunk_w(W, s):
        ws = np.concatenate([W[:, :D], W[:, D:] / s], axis=1)
        wt = ws.T.astype(np.float16)                 # [2D, H]
        return np.ascontiguousarray(
            wt.reshape(KC, P, H).transpose(1, 0, 2).reshape(P, KC * H)
        )

    w1c = chunk_w(W1, S1)
    w2c = chunk_w(W2, S2)
    b1r = np.ascontiguousarray(b1.astype(np.float16).reshape(1, H))
    b2r = np.ascontiguousarray(b2.astype(np.float16).reshape(1, H))

    in_maps = []
    for core in range(NCORES):
        sl = slice(core * BL, (core + 1) * BL)
        self_ids = np.empty((NBLK, BL), dtype=np.int64)
        nb_ids = np.empty((NBLK, BL, S1), dtype=np.int64)
        self_ids[0] = targets[sl]
        nb_ids[0] = nb1_self[sl]
        for j in range(S2):
            self_ids[1 + j] = nb2[sl][:, j]
            nb_ids[1 + j] = nb1_nb[sl][:, j, :]

        # selfs[p, (b*CH + c)*P + r] = x[self_ids[b, r], c*P + p]
        sarr = x[self_ids].astype(np.float16)        # [NBLK, BL, D]
        selfs = np.ascontiguousarray(
            sarr.reshape(NBLK, BL, CH, P)
                .transpose(3, 0, 2, 1)
                .reshape(P, NBLK * U)
        )
        # nb[b*P + p, ((s*CH + c)*P + r)] = x[nb_ids[b, r, s], c*P + p]
        arr = x[nb_ids].astype(np.float16)           # [NBLK, BL, S1, D]
        nb = np.ascontiguousarray(
            arr.reshape(NBLK, BL, S1, CH, P)
               .transpose(0, 4, 2, 3, 1)
               .reshape(NBLK * P, NBW)
        )
        in_maps.append({
            "selfs": selfs, "nb": nb,
            "w1c": w1c, "w2c": w2c, "b1r": b1r, "b2r": b2r,
        })
    return in_maps


def run(trace=False, **inputs):
    from concourse.bass_utils import run_bass_kernel_spmd

    nc = _get_program()
    in_maps = make_in_maps(**inputs)
    res = run_bass_kernel_spmd(
        nc, in_maps, core_ids=list(range(NCORES)), trace=trace
    )
    out = np.concatenate(
        [np.asarray(r["zT"]) for r in res.results], axis=0
    ).astype(np.float32)
    return out, res


def kernel(**inputs) -> np.ndarray:
    out, _ = run(trace=False, **inputs)
    return out


# revision 62
# speedup vs baseline: 1.0225x; 1.0225x over previous
"""GraphSAGE 2-layer minibatch kernel for 8 TRN2 NeuronCores.

Strategy: data-parallel over the 1024-target batch (128 targets/core).
The host lays out each core's working set as fp16 DRAM streams in
compute order, pre-transposed to feature-major: per block (block 0 =
targets, blocks 1..10 = the s2-major nb2 rows) a self tile
[128 feat-partitions, 2 feat-chunks x 128 rows] and a neighbor slab
[128, 26 slot units x 2 chunks x 128 rows] slot-major (padded with one
zero unit). Each slab moves as two DMAs: the first half over the HWDGE
queues (sync/scalar alternating), the second half as a GPSIMD
software-DGE DMA that accumulates (cce add) into the first, so the DMA
itself performs the first fold level (26 -> 13 units). The remaining
13-unit neighbor mean folds as a 5-op tree of wide contiguous DVE adds
(mean scale folded into pre-scaled fp16 weights).

The SAGE layer runs row-major: the feature-major data chunks are the
matmul's stationary lhsT and the fp16 weights the moving rhs, so
z = cat @ W.T lands as [row-partitions, H-cols] in f32 PSUM, the bias
joins as a rank-1 matmul (ones x b) in the same accumulation group,
and the L2 normalize runs per row on ACT: Relu from PSUM, Square with
accum_out (row sum-of-squares), Sqrt, DVE fast reciprocal, Copy with
per-partition scale. The per-block tail (reciprocal, scale,
agg2-accumulate) is software-pipelined one block behind the bulk work
so the in-order DVE/ACT queues never stall a block's fold/matmul on
the previous block's cross-engine round-trip. Layer 2 consumes
PE-transposed layer-1 outputs (block 0 = self half, running sum of
blocks 1..10 = agg half). All DMAs are issued up-front so the kernel
runs at HBM streaming bandwidth with compute chasing the stream.
"""

import numpy as np

N_NODES = 100000
D = 256
H = 256
B = 1024
S1 = 25
S2 = 10
NCORES = 8
BL = B // NCORES          # 128 rows per core
NBLK = 1 + S2             # 11 blocks of 128 layer-1 rows per core
P = 128
CH = D // P               # 2 feature chunks
U = CH * P                # 256-col slot unit (both chunks)
NSU = S1                  # 25 slot units
NBW = NSU * U             # 6400 neighbor cols per block in DRAM
HBW = NBW // 2            # 3200-col half-slab (one per HWDGE queue)
KC = 4                    # contraction chunks per layer (2*D/P)
HC = H // P               # 2 output-feature chunks

_PROG = None  # cached so repeat calls reuse the built program


def _build_program():
    import concourse.mybir as mybir
    from concourse.bacc import Bacc
    from concourse.masks import make_identity
    from concourse.tile import TileContext

    f32 = mybir.dt.float32
    f16 = mybir.dt.float16
    AF = mybir.ActivationFunctionType
    add_op = mybir.AluOpType.add

    nc = Bacc(trn_type="TRN2")

    self_d = nc.dram_tensor("selfs", (P, NBLK * U), f16, kind="ExternalInput")
    nb_d = nc.dram_tensor("nb", (NBLK * P, NBW), f16, kind="ExternalInput")
    w1c_d = nc.dram_tensor("w1c", (P, KC * H), f16, kind="ExternalInput")
    w2c_d = nc.dram_tensor("w2c", (P, KC * H), f16, kind="ExternalInput")
    b1r_d = nc.dram_tensor("b1r", (1, H), f16, kind="ExternalInput")
    b2r_d = nc.dram_tensor("b2r", (1, H), f16, kind="ExternalInput")
    zT_d = nc.dram_tensor("zT", (P, H), f32, kind="ExternalOutput")

    with TileContext(nc) as tc:
        with (
            tc.tile_pool(name="const", bufs=1) as cpool,
            tc.tile_pool(name="nbsg", bufs=S2 // 2) as nbspool,
            tc.tile_pool(name="scr", bufs=2) as scrpool,
            tc.tile_pool(name="agg", bufs=2) as apool,
            tc.tile_pool(name="zsb", bufs=3) as zpool,
            tc.tile_pool(name="sq", bufs=2) as sqpool,
            tc.tile_pool(name="nrm", bufs=3) as nrmpool,
            tc.tile_pool(name="h1", bufs=1) as h1pool,
            tc.tile_pool(name="mm_ps", bufs=2, space="PSUM") as mmpool,
            tc.tile_pool(name="l2_ps", bufs=1, space="PSUM") as l2pool,
            tc.tile_pool(name="tr_ps", bufs=2, space="PSUM") as trpool,
        ):
            ones16 = cpool.tile([1, P], f16, tag="ones16")
            nc.gpsimd.memset(ones16[:], 1.0)
            eps_sb = cpool.tile([P, 1], f32, tag="eps")
            nc.gpsimd.memset(eps_sb[:], 1e-8)
            ident16 = cpool.tile([P, P], f16, tag="ident16")
            make_identity(nc, ident16[:])

            # ---- stream: slab 0 leads; consts interleave behind it ---------
            # Each slab is split across both HWDGE queues. Weights/selfs are
            # queued behind slab 0 (needed from the first sage, ~10us in);
            # w2/b2 ride mid-stream (needed only at layer 2).
            w1_sb = cpool.tile([P, KC * H], f16, tag="w1")
            w2_sb = cpool.tile([P, KC * H], f16, tag="w2")
            b1_sb = cpool.tile([1, H], f16, tag="b1")
            b2_sb = cpool.tile([1, H], f16, tag="b2")
            selfs_sb = cpool.tile([P, NBLK * U], f16, tag="selfs")

            # block 0 rides alone; blocks 1..10 pair up so two blocks share
            # one fold tree. Four half-slab DMAs per pair, spread over the
            # queues (a few mid-stream halves go to the GPSIMD SWDGE queue).
            # One full-width DMA per slab (bigger transfers run the queues
            # nearer peak rate than half/quarter splits), one queue per
            # block within each pair.
            nb0_t = cpool.tile([P, NBW], f16, tag="nb0")
            nc.sync.dma_start(out=nb0_t[:, 0:HBW], in_=nb_d[0:P, 0:HBW])
            nc.scalar.dma_start(out=nb0_t[:, HBW:NBW], in_=nb_d[0:P, HBW:NBW])
            nc.sync.dma_start(out=w1_sb[:], in_=w1c_d[:])
            nc.scalar.dma_start(out=selfs_sb[:], in_=self_d[:])
            nc.sync.dma_start(out=b1_sb[:], in_=b1r_d[:])

            pairs = []
            for k in range(S2 // 2):  # blocks 1..10 in pairs
                a, b = 1 + 2 * k, 2 + 2 * k
                t = nbspool.tile([P, 2 * NBW], f16, tag="nbp", name=f"nbp{k}")
                e0 = nc.sync if k % 2 == 0 else nc.scalar
                e1 = nc.scalar if k % 2 == 0 else nc.sync
                e0.dma_start(out=t[:, 0:NBW], in_=nb_d[a * P:(a + 1) * P, :])
                e1.dma_start(out=t[:, NBW:2 * NBW],
                             in_=nb_d[b * P:(b + 1) * P, :])
                if k == 0:
                    nc.sync.dma_start(out=w2_sb[:], in_=w2c_d[:])
                    nc.scalar.dma_start(out=b2_sb[:], in_=b2r_d[:])
                pairs.append(((a, b), t))

            h1tT_sb = h1pool.tile([P, H], f16, tag="h1tT")   # block-0, f-major
            agg2_sb = h1pool.tile([P, H], f16, tag="agg2")   # sum blocks 1..10
            agg2T_sb = h1pool.tile([P, H], f16, tag="agg2T")
            z2_sb = h1pool.tile([P, H], f32, tag="z2")
            h1t_sb = h1pool.tile([P, H], f16, tag="h1t")

            def fold(slab_t, nblks, split_op1=False):
                """Sum 25 slot units (256 cols each) per block with a wide
                DVE tree; nblks blocks fold in one tree via 3D APs (the
                inner runs stay >= 3072 cols, on the DVE fast path).
                Returns agg [P, nblks*U] feature-major, block j at
                [:, j*U:(j+1)*U]."""
                s3 = slab_t.rearrange("p (b w) -> p b w", b=nblks)
                u = lambda a, b: s3[:, :, a * U:b * U]
                scr = scrpool.tile([P, nblks * 12 * U], f16, tag="scr")
                c3 = scr[:].rearrange("p (b w) -> p b w", b=nblks)
                c = lambda a, b: c3[:, :, a * U:b * U]
                if split_op1:
                    # per-block first level: block j's half starts as soon
                    # as its own slab lands (the stream's last arrivals)
                    for j in range(nblks):
                        nc.vector.tensor_tensor(
                            out=c3[:, j:j + 1, 0:12 * U],
                            in0=s3[:, j:j + 1, 0:12 * U],
                            in1=s3[:, j:j + 1, 12 * U:24 * U],
                            op=add_op)
                else:
                    nc.vector.tensor_tensor(out=c(0, 12), in0=u(0, 12),
                                            in1=u(12, 24), op=add_op)
                nc.vector.tensor_tensor(out=c(0, 6), in0=c(0, 6),
                                        in1=c(6, 12), op=add_op)
                nc.vector.tensor_tensor(out=c(0, 3), in0=c(0, 3),
                                        in1=c(3, 6), op=add_op)
                agg_t = apool.tile([P, nblks * U], f16, tag="agg")
                a3 = agg_t[:].rearrange("p (b w) -> p b w", b=nblks)
                nc.vector.tensor_tensor(out=a3, in0=c(0, 1),
                                        in1=c(1, 2), op=add_op)
                nc.vector.tensor_tensor(out=a3, in0=a3,
                                        in1=c(2, 3), op=add_op)
                nc.vector.tensor_tensor(out=a3, in0=a3,
                                        in1=u(24, 25), op=add_op)
                return agg_t

            def sage_front(cat_chunks, w_sb, b_sb):
                """Bulk of the row-major SAGE layer: matmuls + Relu + row
                sum-of-squares + Sqrt. Returns (z_sb, n_t)."""
                z_ps = mmpool.tile([P, H], f32, space="PSUM", tag="mm")
                for k in range(KC):
                    nc.tensor.matmul(
                        out=z_ps[:],
                        lhsT=cat_chunks[k],
                        rhs=w_sb[:, k * H:(k + 1) * H],
                        start=(k == 0),
                        stop=False,
                    )
                # bias as a rank-1 accumulate: ones(rows) x b
                nc.tensor.matmul(
                    out=z_ps[:], lhsT=ones16[:], rhs=b_sb[:],
                    start=False, stop=True,
                )
                z_sb = zpool.tile([P, H], f16, tag="z")
                nc.scalar.activation(out=z_sb[:], in_=z_ps[:], func=AF.Relu)
                sq_sb = sqpool.tile([P, H], f16, tag="sq")
                ss_t = nrmpool.tile([P, 1], f32, tag="ss")
                nc.scalar.activation(out=sq_sb[:], in_=z_sb[:], func=AF.Square,
                                     accum_out=ss_t[:])
                # inv = (ssq + eps)^-1/2 in a single ACT op; eps keeps
                # all-zero rows finite (z * 1/n = 0 * 1e4 = 0).
                inv = nrmpool.tile([P, 1], f32, tag="inv")
                nc.scalar.activation(out=inv[:], in_=ss_t[:],
                                     func=AF.Abs_reciprocal_sqrt,
                                     bias=eps_sb[:])
                return z_sb, inv

            def sage_tail(src, z_sb, inv):
                """Deferred per-block tail: scale into the block's
                destination (+ agg2 accumulate / h1t transposes)."""
                if src == 0:
                    nc.scalar.activation(out=h1t_sb[:], in_=z_sb[:],
                                         func=AF.Copy, scale=inv[:])
                    transpose2(h1tT_sb, h1t_sb)  # early, off the tail
                elif src == 1:
                    nc.scalar.activation(out=agg2_sb[:], in_=z_sb[:],
                                         func=AF.Copy, scale=inv[:])
                else:
                    hn_t = zpool.tile([P, H], f16, tag="hn")
                    nc.scalar.activation(out=hn_t[:], in_=z_sb[:],
                                         func=AF.Copy, scale=inv[:])
                    nc.vector.tensor_tensor(
                        out=agg2_sb[:], in0=agg2_sb[:], in1=hn_t[:],
                        op=add_op,
                    )

            def transpose2(dst_sb, src_sb):
                """PE-transpose both [P, P] chunks of a row-major [P, H] tile
                into the feature-major dst."""
                for c in range(HC):
                    tr_ps = trpool.tile([P, P], f16, space="PSUM", tag="tr")
                    nc.tensor.transpose(
                        out=tr_ps[:],
                        in_=src_sb[:, c * P:(c + 1) * P],
                        identity=ident16[:],
                    )
                    nc.scalar.copy(dst_sb[:, c * P:(c + 1) * P], tr_ps[:])

            # ---- layer 1: blocks 0..10, tail skewed one block back ---------
            pend = None

            def do_block(src, agg_t, off):
                nonlocal pend
                cat = [
                    selfs_sb[:, src * U:src * U + P],
                    selfs_sb[:, src * U + P:(src + 1) * U],
                    agg_t[:, off:off + P],
                    agg_t[:, off + P:off + 2 * P],
                ]
                z_sb, n_t = sage_front(cat, w1_sb, b1_sb)
                if pend is not None:
                    sage_tail(*pend)
                pend = (src, z_sb, n_t)

            l2_ps = l2pool.tile([P, H], f32, space="PSUM", tag="l2")

            do_block(0, fold(nb0_t[:], 1), 0)
            for pi, ((a, b), t) in enumerate(pairs):
                if pi == len(pairs) - 1:
                    # last pair: two fully independent trees, so block 9's
                    # whole fold+sage clears before block 10 even arrives
                    do_block(a, fold(t[:, 0:NBW], 1), 0)
                    do_block(b, fold(t[:, NBW:2 * NBW], 1), 0)
                else:
                    agg_t = fold(t[:], 2, split_op1=True)
                    do_block(a, agg_t, 0)
                    do_block(b, agg_t, U)
                if pi == 1:
                    # pre-accumulate layer 2's self half + bias into a held
                    # PSUM bank, mid-stream (h1tT and w2 are resident by now)
                    for c in range(HC):
                        nc.tensor.matmul(
                            out=l2_ps[:],
                            lhsT=h1tT_sb[:, c * P:(c + 1) * P],
                            rhs=w2_sb[:, c * H:(c + 1) * H],
                            start=(c == 0),
                            stop=False,
                        )
                    nc.tensor.matmul(
                        out=l2_ps[:], lhsT=ones16[:], rhs=b2_sb[:],
                        start=False, stop=False,
                    )
            sage_tail(*pend)

            # ---- layer 2: only the agg half remains ------------------------
            transpose2(agg2T_sb, agg2_sb)
            for c in range(HC):
                nc.tensor.matmul(
                    out=l2_ps[:],
                    lhsT=agg2T_sb[:, c * P:(c + 1) * P],
                    rhs=w2_sb[:, (2 + c) * H:(3 + c) * H],
                    start=False,
                    stop=(c == HC - 1),
                )
            zf_sb = zpool.tile([P, H], f16, tag="z")
            nc.scalar.activation(out=zf_sb[:], in_=l2_ps[:], func=AF.Relu)
            sqf_sb = sqpool.tile([P, H], f16, tag="sq")
            ssf_t = nrmpool.tile([P, 1], f32, tag="ss")
            nc.scalar.activation(out=sqf_sb[:], in_=zf_sb[:], func=AF.Square,
                                 accum_out=ssf_t[:])
            invf = nrmpool.tile([P, 1], f32, tag="inv")
            nc.scalar.activation(out=invf[:], in_=ssf_t[:],
                                 func=AF.Abs_reciprocal_sqrt, bias=eps_sb[:])
            nc.scalar.activation(out=z2_sb[:], in_=zf_sb[:], func=AF.Copy,
                                 scale=invf[:])
            nc.sync.dma_start(out=zT_d[:], in_=z2_sb[:])

    nc.finalize()
    return nc


def _get_program():
    global _PROG
    if _PROG is None:
        _PROG = _build_program()
    return _PROG


def make_in_maps(x, targets, nb1_self, nb2, nb1_nb, W1, b1, W2, b2):
    """Host-side sharding/preprocessing -> per-core input dicts."""
    x = np.ascontiguousarray(np.asarray(x, dtype=np.float32))
    W1 = np.asarray(W1, dtype=np.float32)
    W2 = np.asarray(W2, dtype=np.float32)
    b1 = np.asarray(b1, dtype=np.float32)
    b2 = np.asarray(b2, dtype=np.float32)
    targets = np.asarray(targets).astype(np.int64)
    nb1_self = np.asarray(nb1_self).astype(np.int64)
    nb2 = np.asarray(nb2).astype(np.int64)
    nb1_nb = np.asarray(nb1_nb).astype(np.int64)

    # fold the neighbor-mean scale into the agg half of each weight matrix,
    # pre-chunked to the SBUF layout: w[p, k*H + m] = W.T[k*128 + p, m]
    def chunk_w(W, s):
        ws = np.concatenate([W[:, :D], W[:, D:] / s], axis=1)
        wt = ws.T.astype(np.float16)                 # [2D, H]
        return np.ascontiguousarray(
            wt.reshape(KC, P, H).transpose(1, 0, 2).reshape(P, KC * H)
        )

    w1c = chunk_w(W1, S1)
    w2c = chunk_w(W2, S2)
    b1r = np.ascontiguousarray(b1.astype(np.float16).reshape(1, H))
    b2r = np.ascontiguousarray(b2.astype(np.float16).reshape(1, H))

    in_maps = []
    for core in range(NCORES):
        sl = slice(core * BL, (core + 1) * BL)
        self_ids = np.empty((NBLK, BL), dtype=np.int64)
        nb_ids = np.empty((NBLK, BL, S1), dtype=np.int64)
        self_ids[0] = targets[sl]
        nb_ids[0] = nb1_self[sl]
        for j in range(S2):
            self_ids[1 + j] = nb2[sl][:, j]
            nb_ids[1 + j] = nb1_nb[sl][:, j, :]

        # selfs[p, (b*CH + c)*P + r] = x[self_ids[b, r], c*P + p]
        sarr = x[self_ids].astype(np.float16)        # [NBLK, BL, D]
        selfs = np.ascontiguousarray(
            sarr.reshape(NBLK, BL, CH, P)
                .transpose(3, 0, 2, 1)
                .reshape(P, NBLK * U)
        )
        # nb[b*P + p, ((s*CH + c)*P + r)] = x[nb_ids[b, r, s], c*P + p]
        arr = x[nb_ids].astype(np.float16)           # [NBLK, BL, S1, D]
        nb = np.ascontiguousarray(
            arr.reshape(NBLK, BL, S1, CH, P)
               .transpose(0, 4, 2, 3, 1)
               .reshape(NBLK * P, NBW)
        )
        in_maps.append({
            "selfs": selfs, "nb": nb,
            "w1c": w1c, "w2c": w2c, "b1r": b1r, "b2r": b2r,
        })
    return in_maps


def run(trace=False, **inputs):
    from concourse.bass_utils import run_bass_kernel_spmd

    nc = _get_program()
    in_maps = make_in_maps(**inputs)
    res = run_bass_kernel_spmd(
        nc, in_maps, core_ids=list(range(NCORES)), trace=trace
    )
    out = np.concatenate(
        [np.asarray(r["zT"]) for r in res.results], axis=0
    ).astype(np.float32)
    return out, res


def kernel(**inputs) -> np.ndarray:
    out, _ = run(trace=False, **inputs)
    return out


# revision 63
# speedup vs baseline: 1.0237x; 1.0011x over previous
"""GraphSAGE 2-layer minibatch kernel for 8 TRN2 NeuronCores.

Strategy: data-parallel over the 1024-target batch (128 targets/core).
The host lays out each core's working set as fp16 DRAM streams in
compute order, pre-transposed to feature-major: per block (block 0 =
targets, blocks 1..10 = the s2-major nb2 rows) a self tile
[128 feat-partitions, 2 feat-chunks x 128 rows] and a neighbor slab
[128, 26 slot units x 2 chunks x 128 rows] slot-major (padded with one
zero unit). Each slab moves as two DMAs: the first half over the HWDGE
queues (sync/scalar alternating), the second half as a GPSIMD
software-DGE DMA that accumulates (cce add) into the first, so the DMA
itself performs the first fold level (26 -> 13 units). The remaining
13-unit neighbor mean folds as a 5-op tree of wide contiguous DVE adds
(mean scale folded into pre-scaled fp16 weights).

The SAGE layer runs row-major: the feature-major data chunks are the
matmul's stationary lhsT and the fp16 weights the moving rhs, so
z = cat @ W.T lands as [row-partitions, H-cols] in f32 PSUM, the bias
joins as a rank-1 matmul (ones x b) in the same accumulation group,
and the L2 normalize runs per row on ACT: Relu from PSUM, Square with
accum_out (row sum-of-squares), Sqrt, DVE fast reciprocal, Copy with
per-partition scale. The per-block tail (reciprocal, scale,
agg2-accumulate) is software-pipelined one block behind the bulk work
so the in-order DVE/ACT queues never stall a block's fold/matmul on
the previous block's cross-engine round-trip. Layer 2 consumes
PE-transposed layer-1 outputs (block 0 = self half, running sum of
blocks 1..10 = agg half). All DMAs are issued up-front so the kernel
runs at HBM streaming bandwidth with compute chasing the stream.
"""

import numpy as np

N_NODES = 100000
D = 256
H = 256
B = 1024
S1 = 25
S2 = 10
NCORES = 8
BL = B // NCORES          # 128 rows per core
NBLK = 1 + S2             # 11 blocks of 128 layer-1 rows per core
P = 128
CH = D // P               # 2 feature chunks
U = CH * P                # 256-col slot unit (both chunks)
NSU = S1                  # 25 slot units
NBW = NSU * U             # 6400 neighbor cols per block in DRAM
HBW = NBW // 2            # 3200-col half-slab (one per HWDGE queue)
KC = 4                    # contraction chunks per layer (2*D/P)
HC = H // P               # 2 output-feature chunks

_PROG = None  # cached so repeat calls reuse the built program


def _build_program():
    import concourse.mybir as mybir
    from concourse.bacc import Bacc
    from concourse.masks import make_identity
    from concourse.tile import TileContext

    f32 = mybir.dt.float32
    f16 = mybir.dt.float16
    AF = mybir.ActivationFunctionType
    add_op = mybir.AluOpType.add

    nc = Bacc(trn_type="TRN2")

    self_d = nc.dram_tensor("selfs", (P, NBLK * U), f16, kind="ExternalInput")
    nb_d = nc.dram_tensor("nb", (NBLK * P, NBW), f16, kind="ExternalInput")
    w1c_d = nc.dram_tensor("w1c", (P, KC * H), f16, kind="ExternalInput")
    w2c_d = nc.dram_tensor("w2c", (P, KC * H), f16, kind="ExternalInput")
    b1r_d = nc.dram_tensor("b1r", (1, H), f16, kind="ExternalInput")
    b2r_d = nc.dram_tensor("b2r", (1, H), f16, kind="ExternalInput")
    zT_d = nc.dram_tensor("zT", (P, H), f32, kind="ExternalOutput")

    with TileContext(nc) as tc:
        with (
            tc.tile_pool(name="const", bufs=1) as cpool,
            tc.tile_pool(name="nbsg", bufs=S2 // 2) as nbspool,
            tc.tile_pool(name="scr", bufs=2) as scrpool,
            tc.tile_pool(name="agg", bufs=2) as apool,
            tc.tile_pool(name="zsb", bufs=3) as zpool,
            tc.tile_pool(name="sq", bufs=2) as sqpool,
            tc.tile_pool(name="nrm", bufs=3) as nrmpool,
            tc.tile_pool(name="h1", bufs=1) as h1pool,
            tc.tile_pool(name="mm_ps", bufs=2, space="PSUM") as mmpool,
            tc.tile_pool(name="l2_ps", bufs=1, space="PSUM") as l2pool,
            tc.tile_pool(name="tr_ps", bufs=2, space="PSUM") as trpool,
        ):
            ones16 = cpool.tile([1, P], f16, tag="ones16")
            nc.gpsimd.memset(ones16[:], 1.0)
            eps_sb = cpool.tile([P, 1], f32, tag="eps")
            nc.gpsimd.memset(eps_sb[:], 1e-8)
            ident16 = cpool.tile([P, P], f16, tag="ident16")
            make_identity(nc, ident16[:])

            # ---- stream: slab 0 leads; consts interleave behind it ---------
            # Each slab is split across both HWDGE queues. Weights/selfs are
            # queued behind slab 0 (needed from the first sage, ~10us in);
            # w2/b2 ride mid-stream (needed only at layer 2).
            w1_sb = cpool.tile([P, KC * H], f16, tag="w1")
            w2_sb = cpool.tile([P, KC * H], f16, tag="w2")
            b1_sb = cpool.tile([1, H], f16, tag="b1")
            b2_sb = cpool.tile([1, H], f16, tag="b2")
            selfs_sb = cpool.tile([P, NBLK * U], f16, tag="selfs")

            # block 0 rides alone; blocks 1..10 pair up so two blocks share
            # one fold tree. Four half-slab DMAs per pair, spread over the
            # queues (a few mid-stream halves go to the GPSIMD SWDGE queue).
            # One full-width DMA per slab (bigger transfers run the queues
            # nearer peak rate than half/quarter splits), one queue per
            # block within each pair.
            nb0_t = cpool.tile([P, NBW], f16, tag="nb0")
            nc.sync.dma_start(out=nb0_t[:, 0:HBW], in_=nb_d[0:P, 0:HBW])
            nc.scalar.dma_start(out=nb0_t[:, HBW:NBW], in_=nb_d[0:P, HBW:NBW])
            nc.sync.dma_start(out=w1_sb[:], in_=w1c_d[:])
            nc.scalar.dma_start(out=selfs_sb[:], in_=self_d[:])
            nc.sync.dma_start(out=b1_sb[:], in_=b1r_d[:])

            pairs = []
            for k in range(S2 // 2):  # blocks 1..10 in pairs
                a, b = 1 + 2 * k, 2 + 2 * k
                t = nbspool.tile([P, 2 * NBW], f16, tag="nbp", name=f"nbp{k}")
                e0 = nc.sync if k % 2 == 0 else nc.scalar
                e1 = nc.scalar if k % 2 == 0 else nc.sync
                e0.dma_start(out=t[:, 0:NBW], in_=nb_d[a * P:(a + 1) * P, :])
                e1.dma_start(out=t[:, NBW:2 * NBW],
                             in_=nb_d[b * P:(b + 1) * P, :])
                if k == 0:
                    nc.sync.dma_start(out=w2_sb[:], in_=w2c_d[:])
                    nc.scalar.dma_start(out=b2_sb[:], in_=b2r_d[:])
                pairs.append(((a, b), t))

            h1tT_sb = h1pool.tile([P, H], f16, tag="h1tT")   # block-0, f-major
            agg2_sb = h1pool.tile([P, H], f16, tag="agg2")   # sum blocks 1..10
            agg2T_sb = h1pool.tile([P, H], f16, tag="agg2T")
            z2_sb = h1pool.tile([P, H], f32, tag="z2")
            h1t_sb = h1pool.tile([P, H], f16, tag="h1t")

            def fold(slab_t, nblks, split_op1=False):
                """Sum 25 slot units (256 cols each) per block with a wide
                DVE tree; nblks blocks fold in one tree via 3D APs (the
                inner runs stay >= 3072 cols, on the DVE fast path).
                Returns agg [P, nblks*U] feature-major, block j at
                [:, j*U:(j+1)*U]."""
                s3 = slab_t[:].rearrange("p (b w) -> p b w", b=nblks)
                u = lambda a, b: s3[:, :, a * U:b * U]
                scr = scrpool.tile([P, nblks * 12 * U], f16, tag="scr")
                c3 = scr[:].rearrange("p (b w) -> p b w", b=nblks)
                c = lambda a, b: c3[:, :, a * U:b * U]
                if split_op1:
                    # per-block first level: block j's half starts as soon
                    # as its own slab lands (the stream's last arrivals)
                    for j in range(nblks):
                        nc.vector.tensor_tensor(
                            out=c3[:, j:j + 1, 0:12 * U],
                            in0=s3[:, j:j + 1, 0:12 * U],
                            in1=s3[:, j:j + 1, 12 * U:24 * U],
                            op=add_op)
                else:
                    nc.vector.tensor_tensor(out=c(0, 12), in0=u(0, 12),
                                            in1=u(12, 24), op=add_op)
                nc.vector.tensor_tensor(out=c(0, 6), in0=c(0, 6),
                                        in1=c(6, 12), op=add_op)
                nc.vector.tensor_tensor(out=c(0, 3), in0=c(0, 3),
                                        in1=c(3, 6), op=add_op)
                agg_t = apool.tile([P, nblks * U], f16, tag="agg")
                a3 = agg_t[:].rearrange("p (b w) -> p b w", b=nblks)
                nc.vector.tensor_tensor(out=a3, in0=c(0, 1),
                                        in1=c(1, 2), op=add_op)
                nc.vector.tensor_tensor(out=a3, in0=a3,
                                        in1=c(2, 3), op=add_op)
                nc.vector.tensor_tensor(out=a3, in0=a3,
                                        in1=u(24, 25), op=add_op)
                return agg_t

            def sage_front(cat_chunks, w_sb, b_sb):
                """Bulk of the row-major SAGE layer: matmuls + Relu + row
                sum-of-squares + Sqrt. Returns (z_sb, n_t)."""
                z_ps = mmpool.tile([P, H], f32, space="PSUM", tag="mm")
                for k in range(KC):
                    nc.tensor.matmul(
                        out=z_ps[:],
                        lhsT=cat_chunks[k],
                        rhs=w_sb[:, k * H:(k + 1) * H],
                        start=(k == 0),
                        stop=False,
                    )
                # bias as a rank-1 accumulate: ones(rows) x b
                nc.tensor.matmul(
                    out=z_ps[:], lhsT=ones16[:], rhs=b_sb[:],
                    start=False, stop=True,
                )
                z_sb = zpool.tile([P, H], f16, tag="z")
                nc.scalar.activation(out=z_sb[:], in_=z_ps[:], func=AF.Relu)
                sq_sb = sqpool.tile([P, H], f16, tag="sq")
                ss_t = nrmpool.tile([P, 1], f32, tag="ss")
                nc.scalar.activation(out=sq_sb[:], in_=z_sb[:], func=AF.Square,
                                     accum_out=ss_t[:])
                # inv = (ssq + eps)^-1/2 in a single ACT op; eps keeps
                # all-zero rows finite (z * 1/n = 0 * 1e4 = 0).
                inv = nrmpool.tile([P, 1], f32, tag="inv")
                nc.scalar.activation(out=inv[:], in_=ss_t[:],
                                     func=AF.Abs_reciprocal_sqrt,
                                     bias=eps_sb[:])
                return z_sb, inv

            def sage_tail(src, z_sb, inv):
                """Deferred per-block tail: scale into the block's
                destination (+ agg2 accumulate / h1t transposes)."""
                if src == 0:
                    nc.scalar.activation(out=h1t_sb[:], in_=z_sb[:],
                                         func=AF.Copy, scale=inv[:])
                    transpose2(h1tT_sb, h1t_sb)  # early, off the tail
                elif src == 1:
                    nc.scalar.activation(out=agg2_sb[:], in_=z_sb[:],
                                         func=AF.Copy, scale=inv[:])
                else:
                    hn_t = zpool.tile([P, H], f16, tag="hn")
                    nc.scalar.activation(out=hn_t[:], in_=z_sb[:],
                                         func=AF.Copy, scale=inv[:])
                    nc.vector.tensor_tensor(
                        out=agg2_sb[:], in0=agg2_sb[:], in1=hn_t[:],
                        op=add_op,
                    )

            def transpose2(dst_sb, src_sb):
                """PE-transpose both [P, P] chunks of a row-major [P, H] tile
                into the feature-major dst."""
                for c in range(HC):
                    tr_ps = trpool.tile([P, P], f16, space="PSUM", tag="tr")
                    nc.tensor.transpose(
                        out=tr_ps[:],
                        in_=src_sb[:, c * P:(c + 1) * P],
                        identity=ident16[:],
                    )
                    nc.scalar.copy(dst_sb[:, c * P:(c + 1) * P], tr_ps[:])

            # ---- layer 1: blocks 0..10, tail skewed one block back ---------
            pend = None

            def do_block(src, agg_t, off):
                nonlocal pend
                cat = [
                    selfs_sb[:, src * U:src * U + P],
                    selfs_sb[:, src * U + P:(src + 1) * U],
                    agg_t[:, off:off + P],
                    agg_t[:, off + P:off + 2 * P],
                ]
                z_sb, n_t = sage_front(cat, w1_sb, b1_sb)
                if pend is not None:
                    sage_tail(*pend)
                pend = (src, z_sb, n_t)

            l2_ps = l2pool.tile([P, H], f32, space="PSUM", tag="l2")

            do_block(0, fold(nb0_t, 1), 0)
            for pi, ((a, b), t) in enumerate(pairs):
                agg_t = fold(t, 2, split_op1=True)
                do_block(a, agg_t, 0)
                do_block(b, agg_t, U)
                if pi == 1:
                    # pre-accumulate layer 2's self half + bias into a held
                    # PSUM bank, mid-stream (h1tT and w2 are resident by now)
                    for c in range(HC):
                        nc.tensor.matmul(
                            out=l2_ps[:],
                            lhsT=h1tT_sb[:, c * P:(c + 1) * P],
                            rhs=w2_sb[:, c * H:(c + 1) * H],
                            start=(c == 0),
                            stop=False,
                        )
                    nc.tensor.matmul(
                        out=l2_ps[:], lhsT=ones16[:], rhs=b2_sb[:],
                        start=False, stop=False,
                    )
            sage_tail(*pend)

            # ---- layer 2: only the agg half remains ------------------------
            transpose2(agg2T_sb, agg2_sb)
            for c in range(HC):
                nc.tensor.matmul(
                    out=l2_ps[:],
                    lhsT=agg2T_sb[:, c * P:(c + 1) * P],
                    rhs=w2_sb[:, (2 + c) * H:(3 + c) * H],
                    start=False,
                    stop=(c == HC - 1),
                )
            zf_sb = zpool.tile([P, H], f16, tag="z")
            nc.scalar.activation(out=zf_sb[:], in_=l2_ps[:], func=AF.Relu)
            sqf_sb = sqpool.tile([P, H], f16, tag="sq")
            ssf_t = nrmpool.tile([P, 1], f32, tag="ss")
            nc.scalar.activation(out=sqf_sb[:], in_=zf_sb[:], func=AF.Square,
                                 accum_out=ssf_t[:])
            invf = nrmpool.tile([P, 1], f32, tag="inv")
            nc.scalar.activation(out=invf[:], in_=ssf_t[:],
                                 func=AF.Abs_reciprocal_sqrt, bias=eps_sb[:])
            nc.scalar.activation(out=z2_sb[:], in_=zf_sb[:], func=AF.Copy,
                                 scale=invf[:])
            nc.sync.dma_start(out=zT_d[:], in_=z2_sb[:])

    nc.finalize()
    return nc


def _get_program():
    global _PROG
    if _PROG is None:
        _PROG = _build_program()
    return _PROG


def make_in_maps(x, targets, nb1_self, nb2, nb1_nb, W1, b1, W2, b2):
    """Host-side sharding/preprocessing -> per-core input dicts."""
    x = np.ascontiguousarray(np.asarray(x, dtype=np.float32))
    W1 = np.asarray(W1, dtype=np.float32)
    W2 = np.asarray(W2, dtype=np.float32)
    b1 = np.asarray(b1, dtype=np.float32)
    b2 = np.asarray(b2, dtype=np.float32)
    targets = np.asarray(targets).astype(np.int64)
    nb1_self = np.asarray(nb1_self).astype(np.int64)
    nb2 = np.asarray(nb2).astype(np.int64)
    nb1_nb = np.asarray(nb1_nb).astype(np.int64)

    # fold the neighbor-mean scale into the agg half of each weight matrix,
    # pre-chunked to the SBUF layout: w[p, k*H + m] = W.T[k*128 + p, m]
    def chunk_w(W, s):
        ws = np.concatenate([W[:, :D], W[:, D:] / s], axis=1)
        wt = ws.T.astype(np.float16)                 # [2D, H]
        return np.ascontiguousarray(
            wt.reshape(KC, P, H).transpose(1, 0, 2).reshape(P, KC * H)
        )

    w1c = chunk_w(W1, S1)
    w2c = chunk_w(W2, S2)
    b1r = np.ascontiguousarray(b1.astype(np.float16).reshape(1, H))
    b2r = np.ascontiguousarray(b2.astype(np.float16).reshape(1, H))

    in_maps = []
    for core in range(NCORES):
        sl = slice(core * BL, (core + 1) * BL)
        self_ids = np.empty((NBLK, BL), dtype=np.int64)
        nb_ids = np.empty((NBLK, BL, S1), dtype=np.int64)
        self_ids[0] = targets[sl]
        nb_ids[0] = nb1_self[sl]
        for j in range(S2):
            self_ids[1 + j] = nb2[sl][:, j]
            nb_ids[1 + j] = nb1_nb[sl][:, j, :]

        # selfs[p, (b*CH + c)*P + r] = x[self_ids[b, r], c*P + p]
        sarr = x[self_ids].astype(np.float16)        # [NBLK, BL, D]
        selfs = np.ascontiguousarray(
            sarr.reshape(NBLK, BL, CH, P)
                .transpose(3, 0, 2, 1)
                .reshape(P, NBLK * U)
        )
        # nb[b*P + p, ((s*CH + c)*P + r)] = x[nb_ids[b, r, s], c*P + p]
        arr = x[nb_ids].astype(np.float16)           # [NBLK, BL, S1, D]
        nb = np.ascontiguousarray(
            arr.reshape(NBLK, BL, S1, CH, P)
               .transpose(0, 4, 2, 3, 1)
               .reshape(NBLK * P, NBW)
        )
        in_maps.append({
            "selfs": selfs, "nb": nb,
            "w1c": w1c, "w2c": w2c, "b1r": b1r, "b2r": b2r,
        })
    return in_maps


def run(trace=False, **inputs):
    from concourse.bass_utils import run_bass_kernel_spmd

    nc = _get_program()
    in_maps = make_in_maps(**inputs)
    res = run_bass_kernel_spmd(
        nc, in_maps, core_ids=list(range(NCORES)), trace=trace
    )
    out = np.concatenate(
        [np.asarray(r["zT"]) for r in res.results], axis=0
    ).astype(np.float32)
    return out, res


def kernel(**inputs) -> np.ndarray:
    out, _ = run(trace=False, **inputs)
    return out
